# revision 1
# baseline (speedup 1.0000x reference)
"""EdgeDegreeEmbedding Trainium2 kernel (8 NeuronCores, SPMD, no collectives).

Strategy: shard by TARGET NODE (625 nodes/core). Host sorts edges by target
node and packs each node's first 16 edges into a 16-row "half"; two halves
form a 32-partition-aligned slot, 8 halves form a 128-edge MLP tile with no
padding columns. A node's message sum is computed by 7 PSUM-accumulated
matmuls (one per m-coefficient) whose stationary operand is a [32,128] slice
of the MLP output m0 and whose moving operand is a host-built block-diagonal
wigner slice [32, 98] (envelope/RESCALE pre-folded) - so the edge->node
scatter-add happens inside the PE with no data reshuffling. Nodes with more
than 16 edges spill into overflow halves that the host adds back at the end.
Each core only touches its private node range -> per-core outputs are
disjoint shards, no allreduce.

The rotation result lands transposed [channel, freq]; the host transposes
back. LayerNorm uses bn_stats + a quake-seeded Newton rsqrt (DVE+GpSimd) so
the scalar engine only ever loads the Silu table. The rotation phase of tile
t-1 is emitted during tile t's MLP (skewed pipeline) so the PE never stalls
on the m0 PSUM->SBUF cast.
"""

import numpy as np

import concourse.bass as bass
import concourse.mybir as mybir
from concourse import tile
from concourse.bass_utils import run_bass_kernel_spmd
from concourse.vector_clock import ScopedClock

# ---- problem constants (hardcoded; must match the reference) ----
SPHERE = 128
M0 = 7
LFULL = 49
CUTOFF = 12.0
RESCALE = 23.395238876342773
LN_EPS = 1e-5
N_NODES, N_EDGES, D_DIST = 5000, 50000, 512

N_CORES = 8
NODES_PER_CORE = N_NODES // N_CORES  # 625
HALF = 16                 # edges per node-half (one node's main capacity)
NPT = 8                   # halves (nodes) per tile
TILE_E = HALF * NPT       # 128 edges per tile, fully dense
H_MAIN = 632              # 625 real nodes + 7 dummies -> multiple of 8
T_MAIN = H_MAIN // NPT    # 79
WCOLS = M0 * 2 * LFULL    # 686: block-diagonal wigner section per tile row
XWF = 6 * 128 + WCOLS     # 768 + 686 = 1454
OUTF = NPT * LFULL        # 392
RMAGIC = 0x5F3759DF

BF16 = mybir.dt.bfloat16
F32 = mybir.dt.float32
I32 = mybir.dt.int32
NP_BF16 = mybir.dt.np(BF16)

_CACHE = {}
TRACE = False      # set True (e.g. from test.py) to profile the run
TRACE_KW = {}      # extra kwargs for run_bass_kernel_spmd when tracing
LAST = None        # BassKernelResults of the most recent run


class _ChunkedDrainTC(tile.TileContext):
    """Walrus here rejects >1 sync wait per instruction; spread every
    multi-wait instruction's extras over preceding same-engine nops, and do
    the same for the Tile exit-drain's global-clock waits."""

    def _lower_ordered_insts(self, ordered):
        for bb_name, insts in ordered.items():
            out = []
            for inst in insts:
                si = getattr(inst, "sync_info", None)
                waits = list(si.on_wait) if si is not None and si.on_wait else []
                if len(waits) > 1 and type(inst).__name__.startswith("Inst"):
                    for w in waits[:-1]:
                        out.append(mybir.InstNoOp(
                            name=self.nc.get_next_instruction_name(),
                            sync_info=mybir.SyncInfo(on_wait=[w], on_update=[]),
                            bass_nofuse=True,
                            engine=inst.engine,
                        ))
                    si.on_wait = waits[-1:]
                out.append(inst)
            ordered[bb_name] = out
        return super()._lower_ordered_insts(ordered)

    def _drain_and_barrier(self, tick_clock, wait_clock):
        nc = self.nc
        probe = nc.sync.nop()
        wait_clock.add_sem_waits(
            probe.ins, ScopedClock({None: tick_clock.global_clock})
        )
        si = probe.ins.sync_info
        waits = list(si.on_wait) if si and si.on_wait else []
        si.on_wait = waits[:1]
        for w in waits[1:]:
            n2 = nc.sync.nop()
            n2.ins.sync_info = mybir.SyncInfo(on_wait=[w], on_update=[])
        nc.sync.drain()
        nc.all_engine_barrier()
        popped = nc._tile_sem_poison_stack.pop()
        assert popped is self._sem_poison
        nc.clear_and_free_semaphores(list(self.sems.allocated().values()))
        nc.all_engine_barrier()


def _build_nc(T):
    """Build the SPMD Bass program for T tiles total (T_MAIN main tiles plus
    T-T_MAIN overflow tiles)."""
    T_OV = T - T_MAIN
    nc = bass.Bass("TRN2", target_bir_lowering=False, num_devices=N_CORES)

    xw = nc.dram_tensor("xw", [T, 128, XWF], BF16, kind="ExternalInput")
    xr = nc.dram_tensor("xr", [T_MAIN, 128, OUTF], F32, kind="ExternalInput")
    w1 = nc.dram_tensor("w1", [128, 6 * 128], BF16, kind="ExternalInput")
    w2 = nc.dram_tensor("w2", [128, 128], BF16, kind="ExternalInput")
    w3 = nc.dram_tensor("w3", [128, M0 * SPHERE], BF16, kind="ExternalInput")
    ident = nc.dram_tensor("ident", [128, 128], BF16, kind="ExternalInput")

    outr = nc.dram_tensor("outr", [T_MAIN, 128, OUTF], F32,
                          kind="ExternalOutput")
    ovr = nc.dram_tensor("ovr", [T_OV, 128, OUTF], F32, kind="ExternalOutput")

    with _ChunkedDrainTC(nc) as tc:
        with (
            tc.tile_pool(name="const", bufs=1) as cpool,
            tc.tile_pool(name="xw", bufs=8) as xw_pool,
            tc.tile_pool(name="xt", bufs=8) as x_pool,
            tc.tile_pool(name="h", bufs=3) as h_pool,
            tc.tile_pool(name="m0", bufs=3) as m0_pool,
            tc.tile_pool(name="outt", bufs=3) as out_pool,
            tc.tile_pool(name="stat", bufs=5) as stat_pool,
            tc.tile_pool(name="ps", bufs=3, space="PSUM") as ps_pool,
            tc.tile_pool(name="psx", bufs=3, space="PSUM") as psx_pool,
            tc.tile_pool(name="psr", bufs=2, space="PSUM") as psr_pool,
        ):
            w1_sb = cpool.tile([128, 6 * 128], BF16)
            nc.sync.dma_start(w1_sb[:], w1[:])
            w2_sb = cpool.tile([128, 128], BF16)
            nc.sync.dma_start(w2_sb[:], w2[:])
            w3_sb = cpool.tile([128, M0 * SPHERE], BF16)
            nc.sync.dma_start(w3_sb[:], w3[:])
            id_sb = cpool.tile([128, 128], BF16)
            nc.sync.dma_start(id_sb[:], ident[:])
            zero1 = cpool.tile([128, 1], F32)
            nc.vector.memset(zero1[:], 0.0)

            def layernorm_silu(ps, h_out):
                """h_out = silu(LN(ps)); ps is a [128,128] f32 psum view.
                rsqrt via quake-seeded Newton split over DVE+GpSimd so the
                ACT table stays on Silu."""
                st = stat_pool.tile([128, 6], F32, tag="bn")
                nc.vector.bn_stats(st[:], ps)
                mv = stat_pool.tile([128, 2], F32, tag="mv")
                nc.vector.bn_aggr(mv[:], st[:])
                ve = stat_pool.tile([128, 1], F32, tag="ve")
                nc.vector.tensor_scalar(ve[:], mv[:, 1:2], LN_EPS, None,
                                        mybir.AluOpType.add)
                yi = stat_pool.tile([128, 1], I32, tag="yi")
                yf = yi[:].bitcast(F32)
                nc.vector.tensor_scalar(yi[:], ve[:].bitcast(I32), 1, None,
                                        mybir.AluOpType.arith_shift_right)
                nc.vector.tensor_scalar(yi[:], yi[:], -1, RMAGIC,
                                        mybir.AluOpType.mult,
                                        mybir.AluOpType.add)
                t1 = stat_pool.tile([128, 1], F32, tag="t1")
                for _ in range(2):
                    nc.gpsimd.tensor_mul(t1[:], yf, yf)
                    nc.gpsimd.tensor_mul(t1[:], t1[:], ve[:])
                    nc.vector.tensor_scalar(t1[:], t1[:], -0.5, 1.5,
                                            mybir.AluOpType.mult,
                                            mybir.AluOpType.add)
                    nc.gpsimd.tensor_mul(yf, yf, t1[:])
                nm = stat_pool.tile([128, 1], F32, tag="nm")
                nc.gpsimd.tensor_mul(nm[:], mv[:, 0:1], yf)
                nc.gpsimd.tensor_sub(nm[:], zero1[:], nm[:])
                nc.scalar.activation(h_out[:], ps,
                                     mybir.ActivationFunctionType.Silu,
                                     bias=nm[:], scale=yf)

            def rot_phase(p):
                """Rotation + output for a previously computed tile: per
                32-aligned slot s and m, accumulate
                rotT[c, halfcols] += m0_slice.T @ w_blockdiag."""
                t, xw_t, x_t, m0_sb = p
                is_main = t < T_MAIN
                out_sb = out_pool.tile([128, OUTF], F32)
                for s in range(4):
                    pb = 32 * s
                    rot = psr_pool.tile([128, 98], F32, tag="rot")
                    for m in range(M0):
                        nc.tensor.matmul(
                            rot[:],
                            m0_sb[pb:pb + 32, m * 128:(m + 1) * 128],
                            xw_t[pb:pb + 32, 768 + m * 98:768 + (m + 1) * 98],
                            start=(m == 0), stop=(m == M0 - 1),
                            tile_position=(pb, 0),
                        )
                    if is_main:
                        nc.vector.tensor_add(out_sb[:, s * 98:(s + 1) * 98],
                                             rot[:], x_t[:, s * 98:(s + 1) * 98])
                    else:
                        nc.vector.tensor_copy(out_sb[:, s * 98:(s + 1) * 98],
                                              rot[:])
                nc.sync.dma_start(outr[t] if is_main else ovr[t - T_MAIN],
                                  out_sb[:])

            prev = None
            for t in range(T):
                is_main = t < T_MAIN
                xw_t = xw_pool.tile([128, XWF], BF16)
                nc.sync.dma_start(xw_t[:], xw[t])
                x_t = None
                if is_main:
                    x_t = x_pool.tile([128, OUTF], F32)
                    nc.gpsimd.dma_start(x_t[:], xr[t])

                # MLP layer 1: x_edge @ W1 -> psum [128e, 128ch]
                ps1 = ps_pool.tile([128, 448], F32, tag="ps")
                for k in range(6):
                    nc.tensor.matmul(
                        ps1[:, 0:128],
                        xw_t[:, k * 128:(k + 1) * 128],
                        w1_sb[:, k * 128:(k + 1) * 128],
                        start=(k == 0), stop=(k == 5),
                    )
                h1 = h_pool.tile([128, 128], BF16, tag="h")
                layernorm_silu(ps1[:, 0:128], h1)

                pst1 = ps_pool.tile([128, 128], BF16, tag="ps")
                nc.tensor.transpose(pst1[:], h1[:], id_sb[:])
                h1t = h_pool.tile([128, 128], BF16, tag="ht")
                nc.vector.tensor_copy(h1t[:], pst1[:])

                # MLP layer 2
                ps2 = ps_pool.tile([128, 448], F32, tag="ps")
                nc.tensor.matmul(ps2[:, 0:128], h1t[:], w2_sb[:],
                                 start=True, stop=True)
                h2 = h_pool.tile([128, 128], BF16, tag="h")
                layernorm_silu(ps2[:, 0:128], h2)

                pst2 = ps_pool.tile([128, 128], BF16, tag="ps")
                nc.tensor.transpose(pst2[:], h2[:], id_sb[:])
                h2t = h_pool.tile([128, 128], BF16, tag="ht")
                nc.vector.tensor_copy(h2t[:], pst2[:])

                # MLP layer 3 -> m0 [128e, 896]; cast to bf16 (ACT + DVE)
                m0a = ps_pool.tile([128, 448], F32, tag="ps")
                nc.tensor.matmul(m0a[:], h2t[:], w3_sb[:, 0:448],
                                 start=True, stop=True)
                m0b = ps_pool.tile([128, 448], F32, tag="ps")
                nc.tensor.matmul(m0b[:], h2t[:], w3_sb[:, 448:896],
                                 start=True, stop=True)
                m0_sb = m0_pool.tile([128, M0 * SPHERE], BF16)
                nc.scalar.activation(m0_sb[:, 0:448], m0a[:],
                                     mybir.ActivationFunctionType.Copy)
                nc.vector.tensor_copy(m0_sb[:, 448:896], m0b[:])

                # skewed pipeline: rotation of the PREVIOUS tile runs while
                # this tile's MLP streams, so the PE never waits on m0
                if prev is not None:
                    rot_phase(prev)
                prev = (t, xw_t, x_t, m0_sb)
            rot_phase(prev)

    return nc


def _envelope(d):
    e = 1.0 + (-21.0) * d ** 5 + 35.0 * d ** 6 + (-15.0) * d ** 7
    return np.where(d < 1.0, e, 0.0)


def kernel(**inputs):
    x = np.asarray(inputs["x"], np.float32)
    dist_emb = np.asarray(inputs["edge_distance_embedding"], np.float32)
    src_emb = np.asarray(inputs["source_atom_embedding"], np.float32)
    tgt_emb = np.asarray(inputs["target_atom_embedding"], np.float32)
    edge_distance = np.asarray(inputs["edge_distance"], np.float64)
    edge_index = np.asarray(inputs["edge_index"]).astype(np.int64)
    wigner = np.asarray(inputs["wigner_and_M_mapping_inv"], np.float32)
    W1 = np.asarray(inputs["W1"], np.float32)
    W2 = np.asarray(inputs["W2"], np.float32)
    W3 = np.asarray(inputs["W3"], np.float32)
    # biases/gains are zeros/ones by construction; folded out of the kernel
    for nm, triv in (("b1", 0), ("bt1", 0), ("b2", 0), ("bt2", 0), ("b3", 0),
                     ("g1", 1), ("g2", 1)):
        v = np.asarray(inputs[nm])
        assert np.all(v == triv), f"{nm} not trivial; unsupported fast path"

    srcs, tgts = edge_index[0], edge_index[1]
    scale = (_envelope(edge_distance / CUTOFF) / RESCALE).astype(np.float32)

    order = np.argsort(tgts, kind="stable")
    tsorted = tgts[order]
    starts = np.searchsorted(tsorted, np.arange(N_NODES + 1))

    # ---- build halves per core (a half = <=16 edges of one node) ----
    core_halves = []
    max_ov = 0
    for c in range(N_CORES):
        halves_main = []
        halves_ov = []
        base = c * NODES_PER_CORE
        for nl in range(NODES_PER_CORE):
            eids = order[starts[base + nl]:starts[base + nl + 1]]
            halves_main.append((nl, eids[:HALF]))
            rest = eids[HALF:]
            while len(rest) > 0:
                halves_ov.append((nl, rest[:HALF]))
                rest = rest[HALF:]
        for nl in range(NODES_PER_CORE, H_MAIN):
            halves_main.append((nl, np.empty(0, np.int64)))  # dummy
        core_halves.append((halves_main, halves_ov))
        max_ov = max(max_ov, len(halves_ov))

    H_OV = max(NPT, -(-max_ov // NPT) * NPT)
    H = H_MAIN + H_OV
    T = H // NPT
    E_pad = H * HALF

    if T not in _CACHE:
        _CACHE[T] = _build_nc(T)
    nc = _CACHE[T]

    # ---- shared weight tensors ----
    w1_in = np.ascontiguousarray(
        W1.reshape(6, 128, 128).transpose(1, 0, 2).reshape(128, 6 * 128)
    ).astype(NP_BF16)
    w2_in = W2.astype(NP_BF16)
    w3_in = W3.astype(NP_BF16)
    ident = np.eye(128, dtype=np.float32).astype(NP_BF16)

    in_maps = []
    ov_maps = []
    for c in range(N_CORES):
        halves_main, halves_ov = core_halves[c]
        halves = halves_main + halves_ov + [
            (0, np.empty(0, np.int64))
        ] * (H_OV - len(halves_ov))

        eorder = np.full(E_pad, -1, np.int64)
        for s, (_, eids) in enumerate(halves):
            eorder[s * HALF:s * HALF + len(eids)] = eids
        valid = eorder >= 0
        idx = eorder[valid]

        # xe gather -> [E_pad, 768] -> [T, 128p, 6k*128e]
        xe = np.zeros((E_pad, 768), np.float32)
        xe[valid, :D_DIST] = dist_emb[idx]
        xe[valid, D_DIST:D_DIST + 128] = src_emb[srcs[idx]]
        xe[valid, D_DIST + 128:] = tgt_emb[tgts[idx]]
        xeT = xe.reshape(T, TILE_E, 6, 128).transpose(0, 3, 2, 1)

        # block-diagonal wigner section:
        # xw[t, 32s+16h+i, 768 + m*98 + h*49 + f] = wig[e,f,m]*scale
        wrows = np.zeros((E_pad, M0, LFULL), np.float32)
        wrows[valid] = (
            wigner[idx, :, :M0] * scale[idx][:, None, None]
        ).transpose(0, 2, 1)
        wr5 = wrows.reshape(T, 4, 2, HALF, M0, LFULL)
        wsec = np.zeros((T, 4, 2, HALF, M0, 2, LFULL), np.float32)
        for h in range(2):
            wsec[:, :, h, :, :, h, :] = wr5[:, :, h]
        wsec = wsec.reshape(T, 128, WCOLS)

        xw_in = np.ascontiguousarray(np.concatenate(
            (xeT.reshape(T, 128, 768), wsec), axis=2,
        )).astype(NP_BF16)

        # x shard, transposed per node: [T_MAIN, 128c, 8h*49f]
        xs = np.zeros((H_MAIN, LFULL, 128), np.float32)
        xs[:NODES_PER_CORE] = x[c * NODES_PER_CORE:(c + 1) * NODES_PER_CORE]
        x_in = np.ascontiguousarray(
            xs.transpose(0, 2, 1)                      # [H_MAIN, 128, 49]
            .reshape(T_MAIN, NPT, 128, LFULL)
            .transpose(0, 2, 1, 3)
            .reshape(T_MAIN, 128, OUTF)
        )

        in_maps.append({
            "xw": xw_in, "xr": x_in,
            "w1": w1_in, "w2": w2_in, "w3": w3_in, "ident": ident,
        })
        ov_maps.append([nl for nl, _ in halves_ov])

    global LAST
    res = run_bass_kernel_spmd(
        nc, in_maps, core_ids=list(range(N_CORES)), trace=TRACE, **TRACE_KW
    )
    LAST = res

    out = np.empty((N_NODES, LFULL, SPHERE), np.float32)
    for c in range(N_CORES):
        r = res.results[c]
        # [T_MAIN, 128c, 8, 49] -> [H_MAIN, 49, 128]
        o = np.asarray(r["outr"], np.float32).reshape(
            T_MAIN, 128, NPT, LFULL).transpose(0, 2, 3, 1).reshape(
            H_MAIN, LFULL, 128)
        oc = o[:NODES_PER_CORE]
        ov = np.asarray(r["ovr"], np.float32).reshape(
            -1, 128, NPT, LFULL).transpose(0, 2, 3, 1).reshape(
            -1, LFULL, 128)
        for s, nl in enumerate(ov_maps[c]):
            oc[nl] += ov[s]
        out[c * NODES_PER_CORE:(c + 1) * NODES_PER_CORE] = oc
    return out



# revision 3
# speedup vs baseline: 2.8777x; 2.8777x over previous
"""EdgeDegreeEmbedding Trainium2 kernel (8 NeuronCores, SPMD, no collectives).

Strategy: shard by TARGET NODE (625 nodes/core). Host sorts edges by target
node and packs each node's first 16 edges into a 16-row "half"; two halves
form a 32-partition-aligned slot, 8 halves form a 128-edge MLP tile with no
padding columns. A node's message sum is computed by 7 PSUM-accumulated
matmuls (one per m-coefficient) whose stationary operand is a [32,128] slice
of the MLP output m0 and whose moving operand is a host-built block-diagonal
wigner slice [32, 98] (envelope/RESCALE pre-folded) - so the edge->node
scatter-add happens inside the PE with no data reshuffling. Nodes with more
than 16 edges spill into overflow halves that the host adds back at the end.
Each core only touches its private node range -> per-core outputs are
disjoint shards, no allreduce.

v2: the per-tile MLP->LN->transpose->rotate chain is software-pipelined
across 4 loop iterations (stage map: A(i) | C,D,E(i-1) | F,G,H(i-2) |
R(i-3)) so every tensor-engine instruction depends only on work emitted a
full iteration earlier - no same-iteration cross-engine bubbles. The x
residual add moved to the host (kills the xr DMA + per-tile adds), the
output is bf16 (halves write traffic), and the LN rsqrt Newton iteration
count dropped to 1 (quake seed is 3% accurate; one iteration gives 2e-3).
"""

import numpy as np

import concourse.bass as bass
import concourse.mybir as mybir
from concourse import tile
from concourse.bass_utils import run_bass_kernel_spmd
from concourse.vector_clock import ScopedClock

# ---- problem constants (hardcoded; must match the reference) ----
SPHERE = 128
M0 = 7
LFULL = 49
CUTOFF = 12.0
RESCALE = 23.395238876342773
LN_EPS = 1e-5
N_NODES, N_EDGES, D_DIST = 5000, 50000, 512

N_CORES = 8
NODES_PER_CORE = N_NODES // N_CORES  # 625
HALF = 16                 # edges per node-half (one node's main capacity)
NPT = 8                   # halves (nodes) per tile
TILE_E = HALF * NPT       # 128 edges per tile, fully dense
H_MAIN = 632              # 625 real nodes + 7 dummies -> multiple of 8
T_MAIN = H_MAIN // NPT    # 79
WCOLS = M0 * 2 * LFULL    # 686: block-diagonal wigner section per tile row
XWF = 6 * 128 + WCOLS     # 768 + 686 = 1454
OUTF = NPT * LFULL        # 392
RMAGIC = 0x5F3759DF

BF16 = mybir.dt.bfloat16
F32 = mybir.dt.float32
I32 = mybir.dt.int32
NP_BF16 = mybir.dt.np(BF16)

_CACHE = {}
TRACE = False      # set True (e.g. from test.py) to profile the run
TRACE_KW = {}      # extra kwargs for run_bass_kernel_spmd when tracing
LAST = None        # BassKernelResults of the most recent run


class _ChunkedDrainTC(tile.TileContext):
    """Walrus here rejects >1 sync wait per instruction; spread every
    multi-wait instruction's extras over preceding same-engine nops, and do
    the same for the Tile exit-drain's global-clock waits."""

    def _lower_ordered_insts(self, ordered):
        for bb_name, insts in ordered.items():
            out = []
            for inst in insts:
                si = getattr(inst, "sync_info", None)
                waits = list(si.on_wait) if si is not None and si.on_wait else []
                if len(waits) > 1 and type(inst).__name__.startswith("Inst"):
                    for w in waits[:-1]:
                        out.append(mybir.InstNoOp(
                            name=self.nc.get_next_instruction_name(),
                            sync_info=mybir.SyncInfo(on_wait=[w], on_update=[]),
                            bass_nofuse=True,
                            engine=inst.engine,
                        ))
                    si.on_wait = waits[-1:]
                out.append(inst)
            ordered[bb_name] = out
        return super()._lower_ordered_insts(ordered)

    def _drain_and_barrier(self, tick_clock, wait_clock):
        nc = self.nc
        probe = nc.sync.nop()
        wait_clock.add_sem_waits(
            probe.ins, ScopedClock({None: tick_clock.global_clock})
        )
        si = probe.ins.sync_info
        waits = list(si.on_wait) if si and si.on_wait else []
        si.on_wait = waits[:1]
        for w in waits[1:]:
            n2 = nc.sync.nop()
            n2.ins.sync_info = mybir.SyncInfo(on_wait=[w], on_update=[])
        nc.sync.drain()
        nc.all_engine_barrier()
        popped = nc._tile_sem_poison_stack.pop()
        assert popped is self._sem_poison
        nc.clear_and_free_semaphores(list(self.sems.allocated().values()))
        nc.all_engine_barrier()


def _build_nc(T):
    """Build the SPMD Bass program for T tiles total (T_MAIN main tiles plus
    T-T_MAIN overflow tiles). Software-pipelined: iteration i emits stage A
    (layer-1 matmul + LN1) for tile i, stages C/D/E (transpose, layer-2, LN2)
    for tile i-1, stages F/G/H (transpose, layer-3, m0 cast) for tile i-2 and
    stage R (rotation matmuls + output) for tile i-3, with the four rotation
    slots interleaved through the iteration to spread PSUM/DMA pressure."""
    nc = bass.Bass("TRN2", target_bir_lowering=False, num_devices=N_CORES)

    xw = nc.dram_tensor("xw", [T, 128, XWF], BF16, kind="ExternalInput")
    w1 = nc.dram_tensor("w1", [128, 6 * 128], BF16, kind="ExternalInput")
    w2 = nc.dram_tensor("w2", [128, 128], BF16, kind="ExternalInput")
    w3 = nc.dram_tensor("w3", [128, M0 * SPHERE], BF16, kind="ExternalInput")
    ident = nc.dram_tensor("ident", [128, 128], BF16, kind="ExternalInput")

    outr = nc.dram_tensor("outr", [T, 128, OUTF], BF16, kind="ExternalOutput")

    with _ChunkedDrainTC(nc) as tc:
        with (
            tc.tile_pool(name="const", bufs=1) as cpool,
            tc.tile_pool(name="xw", bufs=8) as xw_pool,
            tc.tile_pool(name="h", bufs=3) as h_pool,
            tc.tile_pool(name="ht", bufs=2) as ht_pool,
            tc.tile_pool(name="m0", bufs=3) as m0_pool,
            tc.tile_pool(name="outt", bufs=3) as out_pool,
            tc.tile_pool(name="stat", bufs=6) as stat_pool,
            tc.tile_pool(name="ps1", bufs=1, space="PSUM") as ps1_pool,
            tc.tile_pool(name="ps2", bufs=1, space="PSUM") as ps2_pool,
            tc.tile_pool(name="pst", bufs=2, space="PSUM") as pst_pool,
            tc.tile_pool(name="m0ps", bufs=1, space="PSUM") as m0ps_pool,
            tc.tile_pool(name="psr", bufs=2, space="PSUM") as psr_pool,
        ):
            w1_sb = cpool.tile([128, 6 * 128], BF16)
            nc.sync.dma_start(w1_sb[:], w1[:])
            w2_sb = cpool.tile([128, 128], BF16)
            nc.sync.dma_start(w2_sb[:], w2[:])
            w3_sb = cpool.tile([128, M0 * SPHERE], BF16)
            nc.sync.dma_start(w3_sb[:], w3[:])
            id_sb = cpool.tile([128, 128], BF16)
            nc.sync.dma_start(id_sb[:], ident[:])
            zero1 = cpool.tile([128, 1], F32)
            nc.vector.memset(zero1[:], 0.0)

            def layernorm_silu(ps, h_out):
                """h_out = silu(LN(ps)); ps is a [128,128] f32 psum view.
                rsqrt via quake-seeded single Newton step split over
                DVE+GpSimd so the ACT table stays on Silu."""
                st = stat_pool.tile([128, 6], F32, tag="bn")
                nc.vector.bn_stats(st[:], ps)
                mv = stat_pool.tile([128, 2], F32, tag="mv")
                nc.vector.bn_aggr(mv[:], st[:])
                ve = stat_pool.tile([128, 1], F32, tag="ve")
                nc.vector.tensor_scalar(ve[:], mv[:, 1:2], LN_EPS, None,
                                        mybir.AluOpType.add)
                yi = stat_pool.tile([128, 1], I32, tag="yi")
                yf = yi[:].bitcast(F32)
                nc.vector.tensor_scalar(yi[:], ve[:].bitcast(I32), 1, None,
                                        mybir.AluOpType.arith_shift_right)
                nc.vector.tensor_scalar(yi[:], yi[:], -1, RMAGIC,
                                        mybir.AluOpType.mult,
                                        mybir.AluOpType.add)
                t1 = stat_pool.tile([128, 1], F32, tag="t1")
                nc.gpsimd.tensor_mul(t1[:], yf, yf)
                nc.gpsimd.tensor_mul(t1[:], t1[:], ve[:])
                nc.vector.tensor_scalar(t1[:], t1[:], -0.5, 1.5,
                                        mybir.AluOpType.mult,
                                        mybir.AluOpType.add)
                nc.gpsimd.tensor_mul(yf, yf, t1[:])
                nm = stat_pool.tile([128, 1], F32, tag="nm")
                nc.gpsimd.tensor_mul(nm[:], mv[:, 0:1], yf)
                nc.gpsimd.tensor_sub(nm[:], zero1[:], nm[:])
                nc.scalar.activation(h_out[:], ps,
                                     mybir.ActivationFunctionType.Silu,
                                     bias=nm[:], scale=yf)

            # live per-tile state, keyed by tile index
            live = {}

            def rot_slot(t, s):
                """One 32-row slot of the rotation for tile t: 7 accumulated
                matmuls into a [128,98] psum, then copy (cast to bf16) into
                the tile's out_sb. Slots alternate scalar/vector for the
                copy to balance engines."""
                st_ = live[t]
                xw_t, m0_sb, out_sb = st_["xw"], st_["m0"], st_["out"]
                pb = 32 * s
                rot = psr_pool.tile([128, 98], F32, tag="rot")
                for m in range(M0):
                    nc.tensor.matmul(
                        rot[:],
                        m0_sb[pb:pb + 32, m * 128:(m + 1) * 128],
                        xw_t[pb:pb + 32, 768 + m * 98:768 + (m + 1) * 98],
                        start=(m == 0), stop=(m == M0 - 1),
                        tile_position=(pb, 0),
                    )
                dst = out_sb[:, s * 98:(s + 1) * 98]
                if s % 2 == 0:
                    nc.scalar.activation(dst, rot[:],
                                         mybir.ActivationFunctionType.Copy)
                else:
                    nc.vector.tensor_copy(dst, rot[:])
                if s == 3:
                    nc.sync.dma_start(outr[t], out_sb[:])

            for i in range(T + 3):
                t0, t1_, t2, t3 = i, i - 1, i - 2, i - 3
                # prefetch: xw DMA issued 2 iterations ahead of first use
                if i == 0:
                    for tp in range(min(3, T)):
                        xw_t = xw_pool.tile([128, XWF], BF16)
                        nc.sync.dma_start(xw_t[:], xw[tp])
                        live[tp] = {"xw": xw_t}
                elif t0 + 2 < T:
                    xw_t = xw_pool.tile([128, XWF], BF16)
                    nc.sync.dma_start(xw_t[:], xw[t0 + 2])
                    live[t0 + 2] = {"xw": xw_t}

                if t0 < T:
                    st_ = live[t0]
                    # stage A: layer-1 matmul  x_edge @ W1 -> ps1 [128e,128c]
                    ps1 = ps1_pool.tile([128, 128], F32, tag="ps1")
                    for k in range(6):
                        nc.tensor.matmul(
                            ps1[:],
                            st_["xw"][:, k * 128:(k + 1) * 128],
                            w1_sb[:, k * 128:(k + 1) * 128],
                            start=(k == 0), stop=(k == 5),
                        )
                    # stage B: LN1 + silu (vector/gpsimd/scalar chain)
                    h1 = h_pool.tile([128, 128], BF16, tag="h1")
                    layernorm_silu(ps1[:], h1)
                    st_["ps1"], st_["h1"] = ps1, h1

                if t3 >= 0:
                    rot_slot(t3, 0)

                if 0 <= t1_ < T:
                    st_ = live[t1_]
                    # stage C: transpose h1 -> h1t
                    pst1 = pst_pool.tile([128, 128], BF16, tag="pst")
                    nc.tensor.transpose(pst1[:], st_["h1"][:], id_sb[:])
                    h1t = ht_pool.tile([128, 128], BF16, tag="h1t")
                    nc.vector.tensor_copy(h1t[:], pst1[:])
                    # stage D: layer-2 matmul
                    ps2 = ps2_pool.tile([128, 128], F32, tag="ps2")
                    nc.tensor.matmul(ps2[:], h1t[:], w2_sb[:],
                                     start=True, stop=True)
                    st_["ps2"] = ps2

                if t3 >= 0:
                    rot_slot(t3, 1)

                if 0 <= t1_ < T:
                    st_ = live[t1_]
                    # stage E: LN2 + silu
                    h2 = h_pool.tile([128, 128], BF16, tag="h2")
                    layernorm_silu(st_["ps2"][:], h2)
                    st_["h2"] = h2

                if 0 <= t2 < T:
                    st_ = live[t2]
                    # stage F: transpose h2 -> h2t
                    pst2 = pst_pool.tile([128, 128], BF16, tag="pst")
                    nc.tensor.transpose(pst2[:], st_["h2"][:], id_sb[:])
                    h2t = ht_pool.tile([128, 128], BF16, tag="h2t")
                    nc.vector.tensor_copy(h2t[:], pst2[:])
                    # stage G: layer-3 matmul -> m0 [128e, 896] in two psums
                    m0a = m0ps_pool.tile([128, 448], F32, tag="m0a")
                    nc.tensor.matmul(m0a[:], h2t[:], w3_sb[:, 0:448],
                                     start=True, stop=True)
                    m0b = m0ps_pool.tile([128, 448], F32, tag="m0b")
                    nc.tensor.matmul(m0b[:], h2t[:], w3_sb[:, 448:896],
                                     start=True, stop=True)
                    st_["m0a"], st_["m0b"] = m0a, m0b

                if t3 >= 0:
                    rot_slot(t3, 2)

                if 0 <= t2 < T:
                    st_ = live[t2]
                    # stage H: cast m0 psum -> bf16 sbuf (scalar engine)
                    m0_sb = m0_pool.tile([128, M0 * SPHERE], BF16)
                    nc.scalar.activation(m0_sb[:, 0:448], st_["m0a"][:],
                                         mybir.ActivationFunctionType.Copy)
                    nc.scalar.activation(m0_sb[:, 448:896], st_["m0b"][:],
                                         mybir.ActivationFunctionType.Copy)
                    st_["m0"] = m0_sb
                    st_["out"] = out_pool.tile([128, OUTF], BF16,
                                               name="out_sb", tag="out_sb")

                if t3 >= 0:
                    rot_slot(t3, 3)
                    del live[t3]

    return nc


def _envelope(d):
    e = 1.0 + (-21.0) * d ** 5 + 35.0 * d ** 6 + (-15.0) * d ** 7
    return np.where(d < 1.0, e, 0.0)


def kernel(**inputs):
    x = np.asarray(inputs["x"], np.float32)
    dist_emb = np.asarray(inputs["edge_distance_embedding"], np.float32)
    src_emb = np.asarray(inputs["source_atom_embedding"], np.float32)
    tgt_emb = np.asarray(inputs["target_atom_embedding"], np.float32)
    edge_distance = np.asarray(inputs["edge_distance"], np.float64)
    edge_index = np.asarray(inputs["edge_index"]).astype(np.int64)
    wigner = np.asarray(inputs["wigner_and_M_mapping_inv"], np.float32)
    W1 = np.asarray(inputs["W1"], np.float32)
    W2 = np.asarray(inputs["W2"], np.float32)
    W3 = np.asarray(inputs["W3"], np.float32)
    # biases/gains are zeros/ones by construction; folded out of the kernel
    for nm, triv in (("b1", 0), ("bt1", 0), ("b2", 0), ("bt2", 0), ("b3", 0),
                     ("g1", 1), ("g2", 1)):
        v = np.asarray(inputs[nm])
        assert np.all(v == triv), f"{nm} not trivial; unsupported fast path"

    srcs, tgts = edge_index[0], edge_index[1]
    scale = (_envelope(edge_distance / CUTOFF) / RESCALE).astype(np.float32)

    order = np.argsort(tgts, kind="stable")
    tsorted = tgts[order]
    starts = np.searchsorted(tsorted, np.arange(N_NODES + 1))

    # ---- build halves per core (a half = <=16 edges of one node) ----
    core_halves = []
    max_ov = 0
    for c in range(N_CORES):
        halves_main = []
        halves_ov = []
        base = c * NODES_PER_CORE
        for nl in range(NODES_PER_CORE):
            eids = order[starts[base + nl]:starts[base + nl + 1]]
            halves_main.append((nl, eids[:HALF]))
            rest = eids[HALF:]
            while len(rest) > 0:
                halves_ov.append((nl, rest[:HALF]))
                rest = rest[HALF:]
        for nl in range(NODES_PER_CORE, H_MAIN):
            halves_main.append((nl, np.empty(0, np.int64)))  # dummy
        core_halves.append((halves_main, halves_ov))
        max_ov = max(max_ov, len(halves_ov))

    H_OV = max(NPT, -(-max_ov // NPT) * NPT)
    H = H_MAIN + H_OV
    T = H // NPT
    E_pad = H * HALF

    if T not in _CACHE:
        _CACHE[T] = _build_nc(T)
    nc = _CACHE[T]

    # ---- shared weight tensors ----
    w1_in = np.ascontiguousarray(
        W1.reshape(6, 128, 128).transpose(1, 0, 2).reshape(128, 6 * 128)
    ).astype(NP_BF16)
    w2_in = W2.astype(NP_BF16)
    w3_in = W3.astype(NP_BF16)
    ident = np.eye(128, dtype=np.float32).astype(NP_BF16)

    in_maps = []
    ov_maps = []
    for c in range(N_CORES):
        halves_main, halves_ov = core_halves[c]
        halves = halves_main + halves_ov + [
            (0, np.empty(0, np.int64))
        ] * (H_OV - len(halves_ov))

        eorder = np.full(E_pad, -1, np.int64)
        for s, (_, eids) in enumerate(halves):
            eorder[s * HALF:s * HALF + len(eids)] = eids
        valid = eorder >= 0
        idx = eorder[valid]

        # xe gather -> [E_pad, 768] -> [T, 128p, 6k*128e]
        xe = np.zeros((E_pad, 768), np.float32)
        xe[valid, :D_DIST] = dist_emb[idx]
        xe[valid, D_DIST:D_DIST + 128] = src_emb[srcs[idx]]
        xe[valid, D_DIST + 128:] = tgt_emb[tgts[idx]]
        xeT = xe.reshape(T, TILE_E, 6, 128).transpose(0, 3, 2, 1)

        # block-diagonal wigner section:
        # xw[t, 32s+16h+i, 768 + m*98 + h*49 + f] = wig[e,f,m]*scale
        wrows = np.zeros((E_pad, M0, LFULL), np.float32)
        wrows[valid] = (
            wigner[idx, :, :M0] * scale[idx][:, None, None]
        ).transpose(0, 2, 1)
        wr5 = wrows.reshape(T, 4, 2, HALF, M0, LFULL)
        wsec = np.zeros((T, 4, 2, HALF, M0, 2, LFULL), np.float32)
        for h in range(2):
            wsec[:, :, h, :, :, h, :] = wr5[:, :, h]
        wsec = wsec.reshape(T, 128, WCOLS)

        xw_in = np.ascontiguousarray(np.concatenate(
            (xeT.reshape(T, 128, 768), wsec), axis=2,
        )).astype(NP_BF16)

        in_maps.append({
            "xw": xw_in,
            "w1": w1_in, "w2": w2_in, "w3": w3_in, "ident": ident,
        })
        ov_maps.append([nl for nl, _ in halves_ov])

    global LAST
    res = run_bass_kernel_spmd(
        nc, in_maps, core_ids=list(range(N_CORES)), trace=TRACE, **TRACE_KW
    )
    LAST = res

    out = np.empty((N_NODES, LFULL, SPHERE), np.float32)
    for c in range(N_CORES):
        r = res.results[c]
        # [T, 128c, 8, 49] -> [H, 49, 128]
        o = np.asarray(r["outr"], np.float32).reshape(
            T, 128, NPT, LFULL).transpose(0, 2, 3, 1).reshape(
            H, LFULL, 128)
        oc = o[:NODES_PER_CORE]
        for s, nl in enumerate(ov_maps[c]):
            oc[nl] += o[H_MAIN + s]
        out[c * NODES_PER_CORE:(c + 1) * NODES_PER_CORE] = (
            x[c * NODES_PER_CORE:(c + 1) * NODES_PER_CORE] + oc
        )
    return out
